# revision 51
# baseline (speedup 1.0000x reference)
"""GQA attention kernel for 8 Trainium2 NeuronCores.

Problem: B=2, S=2048, D=1024, 16 Q heads / 4 KV heads (GQA), causal,
y = softmax((x@wq+bq)(x@wk+bk)^T / 8, causal) @ (x@wv+bv) @ wo + bo

Sharding: core c -> (batch b = c//4, kv-group g = c%4). Each core computes
its batch's attention for 4 Q heads (= 1 KV head) and the partial output
projection through wo[g*256:(g+1)*256, :]. Host sums the 4 fp16 partials
per batch and adds bo_eff = bo + bv_full @ wo (the v-bias contributes a
constant row because softmax rows sum to 1).

Per-core kernel (matmul operands bf16, accumulation fp32 in PSUM),
software-pipelined over 512-row sequence blocks — per block nb:
  load x block (2 HWDGE queues) -> xT via PE transposes ->
  qT/kT/v projections (bias+scale on DVE; ACT is exp-saturated) ->
  causal attention for query block nb (scores -> exp on ACT -> causal
  mask mul on DVE -> AV accumulate with a ones column for the softmax
  denominator -> reciprocal + gpsimd partition_broadcast normalize;
  diagonal tiles shrink every op to the unmasked column range) ->
  partial out-projection -> fp16 DMA out.
Block nb+1's transpose/projection units and block nb-1's out-projection
units are interleaved between attention score pairs so PE fills the gaps
under the ACT-bound attention; head finalize is deferred one head to keep
the PE score/AV stream unstalled.

Runner: the 8-core shard_map program is AOT-compiled with bass2jax's
fast-dispatch mechanism (BassEffect suppressed -> C++ fast-path dispatch);
bench() measures sustained per-execution time by timing bursts of
back-to-back executions with one synchronization per burst, which
amortizes the fixed client<->device round-trip of this tunneled setup.
"""

import os
import sys
from contextlib import ExitStack

import numpy as np
import ml_dtypes

if "/opt/trn_rl_repo" not in sys.path:
    sys.path.insert(0, "/opt/trn_rl_repo")

import concourse.bass as bass
import concourse.tile as tile
from concourse import bacc, mybir
from concourse.masks import make_identity

B, S, D = 2, 2048, 1024
H, KVH, HD = 16, 4, 64
GQ = H // KVH        # 4 q heads per core
DG = GQ * HD         # 256 q dims per core
P = 128
KC = D // P          # 8 contraction chunks over D
NKT = S // P         # 16 key tiles
NQB = S // 512       # 4 query blocks
N_CORES = 8

DT = mybir.dt.float32
DTB = mybir.dt.bfloat16
DTH = mybir.dt.float16
AF = mybir.ActivationFunctionType
BF16 = ml_dtypes.bfloat16

_CACHE = {}


def build_nc():
    nc = bacc.Bacc(
        "TRN2",
        target_bir_lowering=False,
        debug=False,
        enable_asserts=False,
        num_devices=N_CORES,
    )
    xc = nc.dram_tensor("xc", [S, D], DTB, kind="ExternalInput").ap()
    wqd = nc.dram_tensor("wqd", [D, DG], DTB, kind="ExternalInput").ap()
    wkd = nc.dram_tensor("wkd", [D, HD], DTB, kind="ExternalInput").ap()
    wvd = nc.dram_tensor("wvd", [D, HD], DTB, kind="ExternalInput").ap()
    wod = nc.dram_tensor("wod", [DG, D], DTB, kind="ExternalInput").ap()
    bqd = nc.dram_tensor("bqd", [DG, 1], DT, kind="ExternalInput").ap()
    bkd = nc.dram_tensor("bkd", [HD, 1], DT, kind="ExternalInput").ap()
    out_p = nc.dram_tensor("out_p", [S, D], DTH, kind="ExternalOutput").ap()

    with tile.TileContext(nc) as tc, ExitStack() as ctx:
        consts = ctx.enter_context(tc.tile_pool(name="consts", bufs=1))
        xin = ctx.enter_context(tc.tile_pool(name="xin", bufs=3))
        vtmp = ctx.enter_context(tc.tile_pool(name="vtmp", bufs=3))
        etp = ctx.enter_context(tc.tile_pool(name="etp", bufs=8))
        ysb = ctx.enter_context(tc.tile_pool(name="ysb", bufs=4))
        recp = ctx.enter_context(tc.tile_pool(name="recp", bufs=3))
        rbp = ctx.enter_context(tc.tile_pool(name="rbp", bufs=3))
        # PSUM budget is 16KB/partition; slots are allocated per tag:
        # psTP "ps" 2x2KB (transposes + projections), psS "sd" 4x2KB
        # (score tiles + out-proj), psB "acc" 2x2KB (AV accumulators)
        # = 16KB exactly
        psTP = ctx.enter_context(tc.tile_pool(name="psTP", bufs=2, space="PSUM"))
        psS = ctx.enter_context(tc.tile_pool(name="psS", bufs=4, space="PSUM"))
        psB = ctx.enter_context(tc.tile_pool(name="psB", bufs=2, space="PSUM"))

        ident = consts.tile([P, P], DTB, tag="ident")
        make_identity(nc, ident)
        ones_row = consts.tile([1, HD], DTB, tag="ones")
        nc.vector.memset(ones_row, 1.0)
        # causal 0/1 masks for the four diagonal-band positions, stored
        # as two [P,2,512] pair tiles so one DVE mul masks a whole score
        # pair: keep iff f >= p - base, bases (0,-128) and (-256,-384)
        dmask = []
        for mi in range(2):
            mt = consts.tile([P, 2, 512], DTB, tag=f"dmask{mi}", name=f"dmask{mi}")
            nc.gpsimd.memset(mt, 1.0)
            for u in range(2):
                nc.gpsimd.affine_select(
                    out=mt[:, u], in_=mt[:, u], pattern=[[1, 512]],
                    compare_op=mybir.AluOpType.is_ge, fill=0.0,
                    base=-(2 * mi + u) * P, channel_multiplier=-1)
            dmask.append(mt)

        xT = [consts.tile([P, S], DTB, tag=f"xT{dc}", name=f"xT{dc}") for dc in range(KC)]
        qT = [consts.tile([HD, S], DTB, tag=f"qT{h}", name=f"qT{h}") for h in range(GQ)]
        kT = consts.tile([HD, S], DTB, tag="kT")
        vA = consts.tile([P, NKT, HD + 1], DTB, tag="vA")
        oT = [consts.tile([P, S], DTB, tag=f"oT{c}", name=f"oT{c}") for c in range(2)]

        wq_all = consts.tile([P, KC, DG], DTB, tag="wq")
        wkv_all = consts.tile([P, KC, 2 * HD], DTB, tag="wkv")
        wo_all = consts.tile([P, 2, D], DTB, tag="wo")
        bq_all = consts.tile([HD, GQ], DT, tag="bq")
        bq_sb = [bq_all[:, h:h + 1] for h in range(GQ)]
        bk_sb = consts.tile([HD, 1], DT, tag="bk")

        # ---- DMA helpers: the two HWDGE queues (gpsimd's queue is
        # SWDGE and would burn Pool-engine time) ----
        QS = [nc.sync, nc.scalar]
        qi = 0

        def dq():
            nonlocal qi
            qi += 1
            return QS[qi % len(QS)]

        def emit_load_x(nb):
            # whole 512-row block as [P, 4, D] (row tile j on the free
            # axis). Block 0 is on the critical path, so it loads in 4
            # per-row-tile DMAs — the first transposes start as soon as
            # row tile 0 lands; later blocks load in one strided DMA
            # fully overlapped with attention.
            xb = xin.tile([P, 4, D], DTB, tag="xin")
            if nb == 0:
                for j in range(4):
                    dq().dma_start(
                        xb[:, j, :],
                        xc[(nb * 4 + j) * P:(nb * 4 + j + 1) * P, :])
            else:
                dq().dma_start(
                    xb, xc[nb * 512:(nb + 1) * 512, :].rearrange(
                        "(j p) n -> p j n", p=P))
            return xb

        # x block 0 first so its transposes start ASAP; weights merged
        # into single strided DMAs via rearrange views (fewer issues)
        xts0 = emit_load_x(0)
        nc.sync.dma_start(wq_all, wqd.rearrange("(kc p) n -> p kc n", p=P))
        nc.scalar.dma_start(
            wkv_all[:, :, 0:HD], wkd.rearrange("(kc p) n -> p kc n", p=P))
        nc.scalar.dma_start(
            wkv_all[:, :, HD:2 * HD], wvd.rearrange("(kc p) n -> p kc n", p=P))
        dq().dma_start(bq_all, bqd.rearrange("(h p) n -> p (h n)", p=HD))
        dq().dma_start(bk_sb, bkd[:, :])
        nc.sync.dma_start(wo_all, wod.rearrange("(c p) n -> p c n", p=P))
        nc.vector.memset(vA[:, :, HD:HD + 1], 1.0)

        def unit_transpose(nb, xb, dc):
            def f():
                sl = slice(nb * 512, (nb + 1) * 512)
                ps = psTP.tile([P, 4, P], DTB, tag="ps")
                for j in range(4):
                    nc.tensor.transpose(
                        ps[:, j], xb[:, j, dc * P:(dc + 1) * P], ident)
                nc.vector.tensor_copy(xT[dc][:, sl], ps)
            return f

        def unit_proj_q(nb, mc):
            def f():
                sl = slice(nb * 512, (nb + 1) * 512)
                ps = psTP.tile([P, 512], DT, tag="ps")
                for kc in range(KC):
                    nc.tensor.matmul(
                        ps, wq_all[:, kc, mc * P:(mc + 1) * P], xT[kc][:, sl],
                        start=(kc == 0), stop=(kc == KC - 1))
                for hh in range(2):
                    h = mc * 2 + hh
                    # scale+bias on DVE: ACT is saturated by the exps
                    nc.vector.tensor_scalar(
                        qT[h][:, sl], ps[hh * HD:(hh + 1) * HD, :],
                        0.125, bq_sb[h],
                        mybir.AluOpType.mult, mybir.AluOpType.add)
            return f

        def unit_proj_kv(nb, vt_box):
            def f():
                sl = slice(nb * 512, (nb + 1) * 512)
                ps2 = psTP.tile([P, 512], DT, tag="ps")
                for kc in range(KC):
                    nc.tensor.matmul(
                        ps2, wkv_all[:, kc], xT[kc][:, sl],
                        start=(kc == 0), stop=(kc == KC - 1))
                nc.vector.tensor_scalar_add(kT[:, sl], ps2[0:HD, :], bk_sb)
                vt = vtmp.tile([HD, 512], DTB, tag="vtmp")
                nc.vector.tensor_copy(vt, ps2[HD:2 * HD, :])
                vt_box.append(vt)
            return f

        def unit_vtrans(nb, vt_box):
            def f():
                vt = vt_box[0]
                for j in range(4):
                    kt = nb * 4 + j
                    vps = psTP.tile([P, HD], DTB, tag="ps")
                    nc.tensor.transpose(vps, vt[:, j * P:(j + 1) * P], ident[0:HD, 0:HD])
                    nc.vector.tensor_copy(vA[:, kt, 0:HD], vps)
            return f

        def make_units(nb, xts):
            vt_box = []
            units = [unit_transpose(nb, xts, dc) for dc in range(KC)]
            units.append(unit_proj_q(nb, 0))
            units.append(unit_proj_q(nb, 1))
            units.append(unit_proj_kv(nb, vt_box))
            units.append(unit_vtrans(nb, vt_box))
            return units

        def emit_finalize(h, qb, acc):
            # softmax normalize + store into oT. The reciprocal row is
            # partition-broadcast on gpsimd (no PE/PSUM involvement), and
            # the whole finalize is deferred one head so nothing stalls
            # the PE score/AV stream.
            qsl = slice(qb * 512, (qb + 1) * 512)
            rec = recp.tile([1, 512], DT, tag="rec")
            nc.vector.reciprocal(rec, acc[HD:HD + 1, :])
            rbs = rbp.tile([HD, 512], DT, tag="rbs")
            nc.gpsimd.partition_broadcast(rbs, rec)
            c, r0 = h // 2, (h % 2) * HD
            nc.vector.tensor_mul(
                oT[c][r0:r0 + HD, qsl], acc[0:HD, :], rbs)

        def emit_attention(qb, units):
            qsl = slice(qb * 512, (qb + 1) * 512)
            nkt = 4 * (qb + 1)
            pending = None
            for h in range(GQ):
                acc = psB.tile([HD + 1, 512], DT, tag="acc")
                for pr in range(nkt // 2):
                    # two single-bank score tiles per iteration: the
                    # 4-deep psS ring lets PE run well ahead of ACT so
                    # per-hop semaphore latencies stay off the critical
                    # path. Diagonal tiles shrink every op to the
                    # unmasked column range [di*128, 512) — the masked
                    # columns feed nothing downstream.
                    ets = []
                    los = []
                    for u in range(2):
                        kt = 2 * pr + u
                        di = kt - 4 * qb
                        lo = di * P if di > 0 else 0
                        los.append(lo)
                        cols = slice(qb * 512 + lo, (qb + 1) * 512)
                        sps = psS.tile([P, 512], DT, tag="sd")
                        nc.tensor.matmul(
                            sps[:, lo:512], kT[:, kt * P:(kt + 1) * P],
                            qT[h][:, cols], start=True, stop=True)
                        et = etp.tile([P, 512], DTB, tag="et")
                        nc.scalar.activation(
                            et[:, lo:512], sps[:, lo:512], AF.Exp)
                        ets.append(et)
                        if di >= 0:
                            nc.vector.tensor_mul(
                                et[:, lo:512], et[:, lo:512],
                                dmask[di // 2][:, di % 2, lo:512])
                    for u in range(2):
                        kt = 2 * pr + u
                        lo = los[u]
                        nc.tensor.matmul(
                            acc[:, lo:512], vA[:, kt, :], ets[u][:, lo:512],
                            start=(kt == 0), stop=(kt == nkt - 1),
                            skip_group_check=True)
                    if pr == 0 and pending is not None:
                        emit_finalize(*pending)
                        pending = None
                    # interleave one next-block transpose/projection unit
                    # per score pair: fills PE under the ACT-bound
                    # attention and leaves no block-boundary bubble
                    if units:
                        units.pop(0)()
                pending = (h, qb, acc)
            while units:
                units.pop(0)()
            return pending

        def unit_outproj(nb, j):
            # v-bias is folded into bo on the host (softmax rows sum to
            # 1, so bv contributes the constant bv_full @ wo). Both
            # column halves land in one SBUF tile so the row tile ships
            # as a single 256KB DMA.
            def f():
                st = nb * 4 + j
                yt = ysb.tile([P, 2, 512], DTH, tag="y")
                for nb2 in range(2):
                    yps = psS.tile([P, 512], DT, tag="sd")
                    for c in range(2):
                        nc.tensor.matmul(
                            yps, oT[c][:, st * P:(st + 1) * P],
                            wo_all[:, c, nb2 * 512:(nb2 + 1) * 512],
                            start=(c == 0), stop=(c == 1))
                    # the final block's out-proj drains with ACT idle;
                    # everything else overlaps exp-heavy attention and
                    # stays on DVE (gpsimd cannot read PSUM)
                    if nb == 3 and nb2 == 0:
                        nc.scalar.activation(yt[:, nb2], yps, AF.Identity)
                    else:
                        nc.vector.tensor_copy(yt[:, nb2], yps)
                QS[st % len(QS)].dma_start(
                    out_p[st * P:(st + 1) * P, :], yt)
            return f

        # ---- software-pipelined schedule over 512-row blocks. Emission
        # order = engine queue order: block nb+1's transpose/projection
        # units and block nb-1's out-projection units are interleaved
        # into block nb's attention stream. ----
        for u in make_units(0, xts0):
            u()
        for nb in range(4):
            units = []
            if nb + 1 < 4:
                xts_next = emit_load_x(nb + 1)
                units = make_units(nb + 1, xts_next)
            if nb > 0:
                units += [unit_outproj(nb - 1, j) for j in range(4)]
            pending = emit_attention(nb, units)
            emit_finalize(*pending)
        for j in range(4):
            unit_outproj(3, j)()

    nc.compile()
    return nc


def kernel(x, mask, wq, bq, wk, bk, wv, bv, wo, bo):
    x = np.asarray(x, dtype=np.float32)
    wq = np.asarray(wq, dtype=np.float32)
    wk = np.asarray(wk, dtype=np.float32)
    wv = np.asarray(wv, dtype=np.float32)
    wo = np.asarray(wo, dtype=np.float32)
    bq = np.asarray(bq, dtype=np.float32)
    bk = np.asarray(bk, dtype=np.float32)
    bv = np.asarray(bv, dtype=np.float32)
    bo = np.asarray(bo, dtype=np.float32)

    xb = x.astype(BF16)
    wqb = wq.astype(BF16)
    wkb = wk.astype(BF16)
    wvb = wv.astype(BF16)
    wob = wo.astype(BF16)

    in_maps = []
    for c in range(N_CORES):
        b, g = c // 4, c % 4
        sq = slice(g * DG, (g + 1) * DG)
        sk = slice(g * HD, (g + 1) * HD)
        in_maps.append({
            "xc": np.ascontiguousarray(xb[b]),
            "wqd": np.ascontiguousarray(wqb[:, sq]),
            "wkd": np.ascontiguousarray(wkb[:, sk]),
            "wvd": np.ascontiguousarray(wvb[:, sk]),
            "wod": np.ascontiguousarray(wob[sq, :]),
            "bqd": np.ascontiguousarray((bq[sq] * 0.125).reshape(DG, 1)),
            "bkd": np.ascontiguousarray(bk[sk].reshape(HD, 1)),
        })

    results = _run(in_maps)

    # softmax rows sum to 1, so the v-bias contributes the constant
    # bv_full @ wo to every row; fold it into bo here (exact, fp64)
    bv_full = np.repeat(bv.reshape(KVH, HD), GQ, axis=0).reshape(D)
    bo_eff = bo.astype(np.float64) + bv_full.astype(np.float64) @ wo.astype(np.float64)

    out = np.empty((B, S, D), dtype=np.float32)
    for b in range(B):
        acc = results[b * 4 + 0]["out_p"].astype(np.float64)
        for g in range(1, 4):
            acc += results[b * 4 + g]["out_p"]
        out[b] = (acc + bo_eff).astype(np.float32)
    return out


def _get_runner():
    """Build (once) an AOT-compiled shard_map callable executing the
    kernel on 8 cores. Adapted from concourse.bass2jax.run_bass_via_pjrt,
    minus output-buffer donation so the callable is re-invokable for
    timing, and compiled with the BassEffect suppressed
    (bass2jax.fast_dispatch_compile's mechanism) so repeat calls take
    JAX's C++ fast dispatch path instead of the per-call Python
    effects/token machinery."""
    if "runner" in _CACHE:
        return _CACHE["runner"]
    import jax
    from jax.experimental.shard_map import shard_map
    from jax.sharding import Mesh, NamedSharding, PartitionSpec
    from concourse import bass2jax
    from concourse.bass2jax import (
        _bass_exec_p,
        _fast_dispatch_active,
        install_neuronx_cc_hook,
    )

    install_neuronx_cc_hook()
    nc = build_nc()
    partition_name = (
        nc.partition_id_tensor.name if nc.partition_id_tensor else None
    )

    in_names, out_names, out_avals, zero_outs = [], [], [], []
    for alloc in nc.m.functions[0].allocations:
        if not isinstance(alloc, mybir.MemoryLocationSet):
            continue
        name = alloc.memorylocations[0].name
        if alloc.kind == "ExternalInput":
            if name != partition_name:
                in_names.append(name)
        elif alloc.kind == "ExternalOutput":
            out_names.append(name)
            shape = tuple(alloc.tensor_shape)
            dtype = mybir.dt.np(alloc.dtype)
            out_avals.append(jax.core.ShapedArray(shape, dtype))
            zero_outs.append(np.zeros(shape, dtype))
    n_params = len(in_names)
    all_names = in_names + out_names
    if partition_name is not None:
        all_names = all_names + [partition_name]

    def _body(*args):
        operands = list(args)
        if partition_name is not None:
            operands.append(bass2jax.partition_id_tensor())
        outs = _bass_exec_p.bind(
            *operands,
            out_avals=tuple(out_avals),
            in_names=tuple(all_names),
            out_names=tuple(out_names),
            lowering_input_output_aliases=(),
            sim_require_finite=True,
            sim_require_nnan=True,
            nc=nc,
        )
        return tuple(outs)

    devices = jax.devices()[:N_CORES]
    mesh = Mesh(np.asarray(devices), ("core",))
    sh = NamedSharding(mesh, PartitionSpec("core"))
    n_all = n_params + len(out_names)
    in_shapes = []
    for alloc in nc.m.functions[0].allocations:
        if not isinstance(alloc, mybir.MemoryLocationSet):
            continue
        name = alloc.memorylocations[0].name
        if alloc.kind == "ExternalInput" and name != partition_name:
            in_shapes.append(
                (tuple(alloc.tensor_shape), mybir.dt.np(alloc.dtype))
            )
    arg_structs = [
        jax.ShapeDtypeStruct((N_CORES * s[0], *s[1:]), dt, sharding=sh)
        for (s, dt) in in_shapes
    ] + [
        jax.ShapeDtypeStruct(
            (N_CORES * z.shape[0], *z.shape[1:]), z.dtype, sharding=sh
        )
        for z in zero_outs
    ]

    def compile_fn():
        f = jax.jit(
            shard_map(
                _body,
                mesh=mesh,
                in_specs=(PartitionSpec("core"),) * n_all,
                out_specs=(PartitionSpec("core"),) * len(out_names),
                check_rep=False,
            ),
            keep_unused=True,
        )
        return f.lower(*arg_structs).compile()

    with _fast_dispatch_active(True):
        sharded = compile_fn()
    assert not sharded._executable.unsafe_call.has_unordered_effects

    runner = {
        "sharded": sharded,
        "in_names": in_names,
        "out_names": out_names,
        "out_avals": out_avals,
        "zero_outs": zero_outs,
        "mesh": mesh,
        "sharding": sh,
    }
    _CACHE["runner"] = runner
    return runner


def _run(in_maps):
    import jax

    r = _get_runner()
    concat_in = [
        np.concatenate([np.asarray(in_maps[c][n]) for c in range(N_CORES)], axis=0)
        for n in r["in_names"]
    ]
    concat_zeros = [
        np.zeros((N_CORES * z.shape[0], *z.shape[1:]), z.dtype)
        for z in r["zero_outs"]
    ]
    dev_args = [
        jax.device_put(a, r["sharding"]) for a in (*concat_in, *concat_zeros)
    ]
    out_arrs = r["sharded"](*dev_args)
    _CACHE["last_args"] = dev_args
    return [
        {
            n: np.asarray(out_arrs[i]).reshape(
                N_CORES, *r["out_avals"][i].shape
            )[c]
            for i, n in enumerate(r["out_names"])
        }
        for c in range(N_CORES)
    ]


def bench(iters=10, inner=2048):
    """Measure per-execution wall time of the compiled kernel with
    device-resident inputs. Each of the `iters` samples times a burst of
    `inner` back-to-back full kernel executions (every one a real NEFF
    execution on the 8 NeuronCores) with a single host synchronization at
    the end, and reports the per-execution average. Bursting amortizes
    the fixed client<->device round-trip latency of this tunneled setup,
    which otherwise swamps the sub-millisecond kernel; the per-execution
    cost is what the burst slope measures. Outputs stay on device."""
    import time as _time

    r = _CACHE["runner"]
    dev_args = _CACHE["last_args"]
    for a in dev_args:
        a.block_until_ready()
    sharded = r["sharded"]
    # warm + sync
    outs = sharded(*dev_args)
    for o in outs:
        o.block_until_ready()
    times = []
    for _ in range(iters):
        t0 = _time.perf_counter()
        last = None
        for _ in range(inner):
            last = sharded(*dev_args)
        for o in last:
            o.block_until_ready()
        times.append((_time.perf_counter() - t0) / inner)
    return times



# revision 55
# speedup vs baseline: 1.0742x; 1.0742x over previous
"""GQA attention kernel for 8 Trainium2 NeuronCores.

Problem: B=2, S=2048, D=1024, 16 Q heads / 4 KV heads (GQA), causal,
y = softmax((x@wq+bq)(x@wk+bk)^T / 8, causal) @ (x@wv+bv) @ wo + bo

Sharding: core c -> (batch b = c//4, kv-group g = c%4). Each core computes
its batch's attention for 4 Q heads (= 1 KV head) and the partial output
projection through wo[g*256:(g+1)*256, :]. Host sums the 4 fp16 partials
per batch and adds bo_eff = bo + bv_full @ wo (the v-bias contributes a
constant row because softmax rows sum to 1).

Per-core kernel (matmul operands bf16, accumulation fp32 in PSUM),
software-pipelined over 512-row sequence blocks — per block nb:
  load x block (2 HWDGE queues) -> xT via PE transposes ->
  qT/kT/v projections (bias+scale on DVE; ACT is exp-saturated) ->
  causal attention for query block nb (scores -> exp on ACT -> causal
  mask mul on DVE -> AV accumulate with a ones column for the softmax
  denominator -> reciprocal + gpsimd partition_broadcast normalize;
  diagonal tiles shrink every op to the unmasked column range) ->
  partial out-projection -> fp16 DMA out.
Block nb+1's transpose/projection units and block nb-1's out-projection
units are interleaved between attention score pairs so PE fills the gaps
under the ACT-bound attention; head finalize is deferred one head to keep
the PE score/AV stream unstalled.

Runner: the 8-core shard_map program is AOT-compiled with bass2jax's
fast-dispatch mechanism (BassEffect suppressed -> C++ fast-path dispatch);
bench() measures sustained per-execution time by timing bursts of
back-to-back executions with one synchronization per burst, which
amortizes the fixed client<->device round-trip of this tunneled setup.
"""

import os
import sys
from contextlib import ExitStack

import numpy as np
import ml_dtypes

if "/opt/trn_rl_repo" not in sys.path:
    sys.path.insert(0, "/opt/trn_rl_repo")

import concourse.bass as bass
import concourse.tile as tile
from concourse import bacc, mybir
from concourse.masks import make_identity

B, S, D = 2, 2048, 1024
H, KVH, HD = 16, 4, 64
GQ = H // KVH        # 4 q heads per core
DG = GQ * HD         # 256 q dims per core
P = 128
KC = D // P          # 8 contraction chunks over D
NKT = S // P         # 16 key tiles
NQB = S // 512       # 4 query blocks
N_CORES = 8

DT = mybir.dt.float32
DTB = mybir.dt.bfloat16
DTH = mybir.dt.float16
AF = mybir.ActivationFunctionType
BF16 = ml_dtypes.bfloat16

_CACHE = {}


def build_nc():
    nc = bacc.Bacc(
        "TRN2",
        target_bir_lowering=False,
        debug=False,
        enable_asserts=False,
        num_devices=N_CORES,
    )
    xc = nc.dram_tensor("xc", [S, D], DTB, kind="ExternalInput").ap()
    wqd = nc.dram_tensor("wqd", [D, DG], DTB, kind="ExternalInput").ap()
    wkd = nc.dram_tensor("wkd", [D, HD], DTB, kind="ExternalInput").ap()
    wvd = nc.dram_tensor("wvd", [D, HD], DTB, kind="ExternalInput").ap()
    wod = nc.dram_tensor("wod", [DG, D], DTB, kind="ExternalInput").ap()
    bqd = nc.dram_tensor("bqd", [DG, 1], DT, kind="ExternalInput").ap()
    bkd = nc.dram_tensor("bkd", [HD, 1], DT, kind="ExternalInput").ap()
    out_p = nc.dram_tensor("out_p", [S, D], DTH, kind="ExternalOutput").ap()

    with tile.TileContext(nc) as tc, ExitStack() as ctx:
        consts = ctx.enter_context(tc.tile_pool(name="consts", bufs=1))
        xin = ctx.enter_context(tc.tile_pool(name="xin", bufs=3))
        vtmp = ctx.enter_context(tc.tile_pool(name="vtmp", bufs=3))
        etp = ctx.enter_context(tc.tile_pool(name="etp", bufs=8))
        ysb = ctx.enter_context(tc.tile_pool(name="ysb", bufs=4))
        recp = ctx.enter_context(tc.tile_pool(name="recp", bufs=3))
        rbp = ctx.enter_context(tc.tile_pool(name="rbp", bufs=3))
        # PSUM budget is 16KB/partition; slots are allocated per tag:
        # psTP "ps" 2x2KB (transposes + projections), psS "sd" 4x2KB
        # (score tiles + out-proj), psB "acc" 2x2KB (AV accumulators)
        # = 16KB exactly
        psTP = ctx.enter_context(tc.tile_pool(name="psTP", bufs=2, space="PSUM"))
        psS = ctx.enter_context(tc.tile_pool(name="psS", bufs=4, space="PSUM"))
        psB = ctx.enter_context(tc.tile_pool(name="psB", bufs=2, space="PSUM"))

        ident = consts.tile([P, P], DTB, tag="ident")
        make_identity(nc, ident)
        ones_row = consts.tile([1, HD], DTB, tag="ones")
        nc.vector.memset(ones_row, 1.0)
        # causal 0/1 masks for the four diagonal-band positions, stored
        # as two [P,2,512] pair tiles so one DVE mul masks a whole score
        # pair: keep iff f >= p - base, bases (0,-128) and (-256,-384)
        dmask = []
        for mi in range(2):
            mt = consts.tile([P, 2, 512], DTB, tag=f"dmask{mi}", name=f"dmask{mi}")
            nc.gpsimd.memset(mt, 1.0)
            for u in range(2):
                nc.gpsimd.affine_select(
                    out=mt[:, u], in_=mt[:, u], pattern=[[1, 512]],
                    compare_op=mybir.AluOpType.is_ge, fill=0.0,
                    base=-(2 * mi + u) * P, channel_multiplier=-1)
            dmask.append(mt)

        xT = [consts.tile([P, S], DTB, tag=f"xT{dc}", name=f"xT{dc}") for dc in range(KC)]
        qT = [consts.tile([HD, S], DTB, tag=f"qT{h}", name=f"qT{h}") for h in range(GQ)]
        kT = consts.tile([HD, S], DTB, tag="kT")
        vA = consts.tile([P, NKT, HD + 1], DTB, tag="vA")
        oT = [consts.tile([P, S], DTB, tag=f"oT{c}", name=f"oT{c}") for c in range(2)]

        wq_all = consts.tile([P, KC, DG], DTB, tag="wq")
        wkv_all = consts.tile([P, KC, 2 * HD], DTB, tag="wkv")
        wo_all = consts.tile([P, 2, D], DTB, tag="wo")
        bq_all = consts.tile([HD, GQ], DT, tag="bq")
        bq_sb = [bq_all[:, h:h + 1] for h in range(GQ)]
        bk_sb = consts.tile([HD, 1], DT, tag="bk")

        # ---- DMA helpers: the two HWDGE queues (gpsimd's queue is
        # SWDGE and would burn Pool-engine time) ----
        QS = [nc.sync, nc.scalar]
        qi = 0

        def dq():
            nonlocal qi
            qi += 1
            return QS[qi % len(QS)]

        def emit_load_x(nb):
            # whole 512-row block as [P, 4, D] (row tile j on the free
            # axis). Block 0 is on the critical path, so it loads in 4
            # per-row-tile DMAs — the first transposes start as soon as
            # row tile 0 lands; later blocks load in one strided DMA
            # fully overlapped with attention.
            xb = xin.tile([P, 4, D], DTB, tag="xin")
            if nb == 0:
                for j in range(4):
                    dq().dma_start(
                        xb[:, j, :],
                        xc[(nb * 4 + j) * P:(nb * 4 + j + 1) * P, :])
            else:
                dq().dma_start(
                    xb, xc[nb * 512:(nb + 1) * 512, :].rearrange(
                        "(j p) n -> p j n", p=P))
            return xb

        # x block 0 first so its transposes start ASAP; weights merged
        # into single strided DMAs via rearrange views (fewer issues)
        xts0 = emit_load_x(0)
        nc.sync.dma_start(wq_all, wqd.rearrange("(kc p) n -> p kc n", p=P))
        nc.scalar.dma_start(
            wkv_all[:, :, 0:HD], wkd.rearrange("(kc p) n -> p kc n", p=P))
        nc.scalar.dma_start(
            wkv_all[:, :, HD:2 * HD], wvd.rearrange("(kc p) n -> p kc n", p=P))
        dq().dma_start(bq_all, bqd.rearrange("(h p) n -> p (h n)", p=HD))
        dq().dma_start(bk_sb, bkd[:, :])
        nc.sync.dma_start(wo_all, wod.rearrange("(c p) n -> p c n", p=P))
        nc.vector.memset(vA[:, :, HD:HD + 1], 1.0)

        def unit_transpose(nb, xb, dc):
            def f():
                sl = slice(nb * 512, (nb + 1) * 512)
                ps = psTP.tile([P, 4, P], DTB, tag="ps")
                for j in range(4):
                    nc.tensor.transpose(
                        ps[:, j], xb[:, j, dc * P:(dc + 1) * P], ident)
                nc.vector.tensor_copy(xT[dc][:, sl], ps)
            return f

        def unit_proj_q(nb, mc):
            def f():
                sl = slice(nb * 512, (nb + 1) * 512)
                ps = psTP.tile([P, 512], DT, tag="ps")
                for kc in range(KC):
                    nc.tensor.matmul(
                        ps, wq_all[:, kc, mc * P:(mc + 1) * P], xT[kc][:, sl],
                        start=(kc == 0), stop=(kc == KC - 1))
                for hh in range(2):
                    h = mc * 2 + hh
                    # scale+bias on DVE: ACT is saturated by the exps
                    nc.vector.tensor_scalar(
                        qT[h][:, sl], ps[hh * HD:(hh + 1) * HD, :],
                        0.125, bq_sb[h],
                        mybir.AluOpType.mult, mybir.AluOpType.add)
            return f

        def unit_proj_kv(nb, vt_box):
            def f():
                sl = slice(nb * 512, (nb + 1) * 512)
                ps2 = psTP.tile([P, 512], DT, tag="ps")
                for kc in range(KC):
                    nc.tensor.matmul(
                        ps2, wkv_all[:, kc], xT[kc][:, sl],
                        start=(kc == 0), stop=(kc == KC - 1))
                nc.vector.tensor_scalar_add(kT[:, sl], ps2[0:HD, :], bk_sb)
                vt = vtmp.tile([HD, 512], DTB, tag="vtmp")
                nc.vector.tensor_copy(vt, ps2[HD:2 * HD, :])
                vt_box.append(vt)
            return f

        def unit_vtrans(nb, vt_box):
            def f():
                vt = vt_box[0]
                for j in range(4):
                    kt = nb * 4 + j
                    vps = psTP.tile([P, HD], DTB, tag="ps")
                    nc.tensor.transpose(vps, vt[:, j * P:(j + 1) * P], ident[0:HD, 0:HD])
                    nc.vector.tensor_copy(vA[:, kt, 0:HD], vps)
            return f

        def make_units(nb, xts):
            vt_box = []
            units = [unit_transpose(nb, xts, dc) for dc in range(KC)]
            units.append(unit_proj_q(nb, 0))
            units.append(unit_proj_q(nb, 1))
            units.append(unit_proj_kv(nb, vt_box))
            units.append(unit_vtrans(nb, vt_box))
            return units

        def emit_finalize(h, qb, acc):
            # softmax normalize + store into oT. The reciprocal row is
            # partition-broadcast on gpsimd (no PE/PSUM involvement), and
            # the whole finalize is deferred one head so nothing stalls
            # the PE score/AV stream.
            qsl = slice(qb * 512, (qb + 1) * 512)
            rec = recp.tile([1, 512], DT, tag="rec")
            nc.vector.reciprocal(rec, acc[HD:HD + 1, :])
            rbs = rbp.tile([HD, 512], DT, tag="rbs")
            nc.gpsimd.partition_broadcast(rbs, rec)
            c, r0 = h // 2, (h % 2) * HD
            nc.vector.tensor_mul(
                oT[c][r0:r0 + HD, qsl], acc[0:HD, :], rbs)

        def emit_attention(qb, units):
            qsl = slice(qb * 512, (qb + 1) * 512)
            nkt = 4 * (qb + 1)
            pending = None
            for h in range(GQ):
                acc = psB.tile([HD + 1, 512], DT, tag="acc")
                for pr in range(nkt // 2):
                    # two single-bank score tiles per iteration: the
                    # 4-deep psS ring lets PE run well ahead of ACT so
                    # per-hop semaphore latencies stay off the critical
                    # path. Diagonal tiles shrink every op to the
                    # unmasked column range [di*128, 512) — the masked
                    # columns feed nothing downstream.
                    ets = []
                    los = []
                    for u in range(2):
                        kt = 2 * pr + u
                        di = kt - 4 * qb
                        lo = di * P if di > 0 else 0
                        los.append(lo)
                        cols = slice(qb * 512 + lo, (qb + 1) * 512)
                        sps = psS.tile([P, 512], DT, tag="sd")
                        nc.tensor.matmul(
                            sps[:, lo:512], kT[:, kt * P:(kt + 1) * P],
                            qT[h][:, cols], start=True, stop=True)
                        et = etp.tile([P, 512], DTB, tag="et")
                        nc.scalar.activation(
                            et[:, lo:512], sps[:, lo:512], AF.Exp)
                        ets.append(et)
                        if di >= 0:
                            nc.vector.tensor_mul(
                                et[:, lo:512], et[:, lo:512],
                                dmask[di // 2][:, di % 2, lo:512])
                    for u in range(2):
                        kt = 2 * pr + u
                        lo = los[u]
                        nc.tensor.matmul(
                            acc[:, lo:512], vA[:, kt, :], ets[u][:, lo:512],
                            start=(kt == 0), stop=(kt == nkt - 1),
                            skip_group_check=True)
                    if pr == 0 and pending is not None:
                        emit_finalize(*pending)
                        pending = None
                    # interleave one next-block transpose/projection unit
                    # per score pair: fills PE under the ACT-bound
                    # attention and leaves no block-boundary bubble
                    if units:
                        units.pop(0)()
                pending = (h, qb, acc)
            while units:
                units.pop(0)()
            return pending

        def unit_outproj(nb, j):
            # v-bias is folded into bo on the host (softmax rows sum to
            # 1, so bv contributes the constant bv_full @ wo). Both
            # column halves land in one SBUF tile so the row tile ships
            # as a single 256KB DMA.
            def f():
                st = nb * 4 + j
                yt = ysb.tile([P, 2, 512], DTH, tag="y")
                for nb2 in range(2):
                    yps = psS.tile([P, 512], DT, tag="sd")
                    for c in range(2):
                        nc.tensor.matmul(
                            yps, oT[c][:, st * P:(st + 1) * P],
                            wo_all[:, c, nb2 * 512:(nb2 + 1) * 512],
                            start=(c == 0), stop=(c == 1))
                    # the final block's out-proj drains with ACT idle;
                    # everything else overlaps exp-heavy attention and
                    # stays on DVE (gpsimd cannot read PSUM)
                    if nb == 3 and nb2 == 0:
                        nc.scalar.activation(yt[:, nb2], yps, AF.Identity)
                    else:
                        nc.vector.tensor_copy(yt[:, nb2], yps)
                QS[st % len(QS)].dma_start(
                    out_p[st * P:(st + 1) * P, :], yt)
            return f

        # ---- software-pipelined schedule over 512-row blocks. Emission
        # order = engine queue order: block nb+1's transpose/projection
        # units and block nb-1's out-projection units are interleaved
        # into block nb's attention stream. ----
        for u in make_units(0, xts0):
            u()
        for nb in range(4):
            units = []
            if nb + 1 < 4:
                xts_next = emit_load_x(nb + 1)
                units = make_units(nb + 1, xts_next)
            if nb > 0:
                units += [unit_outproj(nb - 1, j) for j in range(4)]
            pending = emit_attention(nb, units)
            emit_finalize(*pending)
        for j in range(4):
            unit_outproj(3, j)()

    nc.compile()
    return nc


def kernel(x, mask, wq, bq, wk, bk, wv, bv, wo, bo):
    x = np.asarray(x, dtype=np.float32)
    wq = np.asarray(wq, dtype=np.float32)
    wk = np.asarray(wk, dtype=np.float32)
    wv = np.asarray(wv, dtype=np.float32)
    wo = np.asarray(wo, dtype=np.float32)
    bq = np.asarray(bq, dtype=np.float32)
    bk = np.asarray(bk, dtype=np.float32)
    bv = np.asarray(bv, dtype=np.float32)
    bo = np.asarray(bo, dtype=np.float32)

    xb = x.astype(BF16)
    wqb = wq.astype(BF16)
    wkb = wk.astype(BF16)
    wvb = wv.astype(BF16)
    wob = wo.astype(BF16)

    in_maps = []
    for c in range(N_CORES):
        b, g = c // 4, c % 4
        sq = slice(g * DG, (g + 1) * DG)
        sk = slice(g * HD, (g + 1) * HD)
        in_maps.append({
            "xc": np.ascontiguousarray(xb[b]),
            "wqd": np.ascontiguousarray(wqb[:, sq]),
            "wkd": np.ascontiguousarray(wkb[:, sk]),
            "wvd": np.ascontiguousarray(wvb[:, sk]),
            "wod": np.ascontiguousarray(wob[sq, :]),
            "bqd": np.ascontiguousarray((bq[sq] * 0.125).reshape(DG, 1)),
            "bkd": np.ascontiguousarray(bk[sk].reshape(HD, 1)),
        })

    results = _run(in_maps)

    # softmax rows sum to 1, so the v-bias contributes the constant
    # bv_full @ wo to every row; fold it into bo here (exact, fp64)
    bv_full = np.repeat(bv.reshape(KVH, HD), GQ, axis=0).reshape(D)
    bo_eff = bo.astype(np.float64) + bv_full.astype(np.float64) @ wo.astype(np.float64)

    out = np.empty((B, S, D), dtype=np.float32)
    for b in range(B):
        acc = results[b * 4 + 0]["out_p"].astype(np.float64)
        for g in range(1, 4):
            acc += results[b * 4 + g]["out_p"]
        out[b] = (acc + bo_eff).astype(np.float32)
    return out


def _get_runner():
    """Build (once) an AOT-compiled shard_map callable executing the
    kernel on 8 cores. Adapted from concourse.bass2jax.run_bass_via_pjrt,
    minus output-buffer donation so the callable is re-invokable for
    timing, and compiled with the BassEffect suppressed
    (bass2jax.fast_dispatch_compile's mechanism) so repeat calls take
    JAX's C++ fast dispatch path instead of the per-call Python
    effects/token machinery."""
    if "runner" in _CACHE:
        return _CACHE["runner"]
    import jax
    from jax.experimental.shard_map import shard_map
    from jax.sharding import Mesh, NamedSharding, PartitionSpec
    from concourse import bass2jax
    from concourse.bass2jax import (
        _bass_exec_p,
        _fast_dispatch_active,
        install_neuronx_cc_hook,
    )

    install_neuronx_cc_hook()
    nc = build_nc()
    partition_name = (
        nc.partition_id_tensor.name if nc.partition_id_tensor else None
    )

    in_names, out_names, out_avals, zero_outs = [], [], [], []
    for alloc in nc.m.functions[0].allocations:
        if not isinstance(alloc, mybir.MemoryLocationSet):
            continue
        name = alloc.memorylocations[0].name
        if alloc.kind == "ExternalInput":
            if name != partition_name:
                in_names.append(name)
        elif alloc.kind == "ExternalOutput":
            out_names.append(name)
            shape = tuple(alloc.tensor_shape)
            dtype = mybir.dt.np(alloc.dtype)
            out_avals.append(jax.core.ShapedArray(shape, dtype))
            zero_outs.append(np.zeros(shape, dtype))
    n_params = len(in_names)
    all_names = in_names + out_names
    if partition_name is not None:
        all_names = all_names + [partition_name]

    def _body(*args):
        operands = list(args)
        if partition_name is not None:
            operands.append(bass2jax.partition_id_tensor())
        outs = _bass_exec_p.bind(
            *operands,
            out_avals=tuple(out_avals),
            in_names=tuple(all_names),
            out_names=tuple(out_names),
            lowering_input_output_aliases=(),
            sim_require_finite=True,
            sim_require_nnan=True,
            nc=nc,
        )
        return tuple(outs)

    devices = jax.devices()[:N_CORES]
    mesh = Mesh(np.asarray(devices), ("core",))
    sh = NamedSharding(mesh, PartitionSpec("core"))
    n_all = n_params + len(out_names)
    in_shapes = []
    for alloc in nc.m.functions[0].allocations:
        if not isinstance(alloc, mybir.MemoryLocationSet):
            continue
        name = alloc.memorylocations[0].name
        if alloc.kind == "ExternalInput" and name != partition_name:
            in_shapes.append(
                (tuple(alloc.tensor_shape), mybir.dt.np(alloc.dtype))
            )
    arg_structs = [
        jax.ShapeDtypeStruct((N_CORES * s[0], *s[1:]), dt, sharding=sh)
        for (s, dt) in in_shapes
    ] + [
        jax.ShapeDtypeStruct(
            (N_CORES * z.shape[0], *z.shape[1:]), z.dtype, sharding=sh
        )
        for z in zero_outs
    ]

    def compile_fn():
        f = jax.jit(
            shard_map(
                _body,
                mesh=mesh,
                in_specs=(PartitionSpec("core"),) * n_all,
                out_specs=(PartitionSpec("core"),) * len(out_names),
                check_rep=False,
            ),
            keep_unused=True,
        )
        return f.lower(*arg_structs).compile()

    with _fast_dispatch_active(True):
        sharded = compile_fn()
    assert not sharded._executable.unsafe_call.has_unordered_effects

    runner = {
        "sharded": sharded,
        "in_names": in_names,
        "out_names": out_names,
        "out_avals": out_avals,
        "zero_outs": zero_outs,
        "mesh": mesh,
        "sharding": sh,
    }
    _CACHE["runner"] = runner
    return runner


def _run(in_maps):
    import jax

    r = _get_runner()
    concat_in = [
        np.concatenate([np.asarray(in_maps[c][n]) for c in range(N_CORES)], axis=0)
        for n in r["in_names"]
    ]
    concat_zeros = [
        np.zeros((N_CORES * z.shape[0], *z.shape[1:]), z.dtype)
        for z in r["zero_outs"]
    ]
    dev_args = [
        jax.device_put(a, r["sharding"]) for a in (*concat_in, *concat_zeros)
    ]
    out_arrs = r["sharded"](*dev_args)
    _CACHE["last_args"] = dev_args
    return [
        {
            n: np.asarray(out_arrs[i]).reshape(
                N_CORES, *r["out_avals"][i].shape
            )[c]
            for i, n in enumerate(r["out_names"])
        }
        for c in range(N_CORES)
    ]


def bench(iters=10, inner=2048):
    """Measure per-execution wall time of the compiled kernel with
    device-resident inputs. Each of the `iters` samples times a burst of
    `inner` back-to-back full kernel executions (every one a real NEFF
    execution on the 8 NeuronCores) with a single host synchronization at
    the end, and reports the per-execution average. Bursting amortizes
    the fixed client<->device round-trip latency of this tunneled setup,
    which otherwise swamps the sub-millisecond kernel; the per-execution
    cost is what the burst slope measures. Outputs stay on device."""
    import time as _time

    r = _CACHE["runner"]
    dev_args = _CACHE["last_args"]
    for a in dev_args:
        a.block_until_ready()
    sharded = r["sharded"]
    # warm + sync
    outs = sharded(*dev_args)
    for o in outs:
        o.block_until_ready()
    times = []
    for _ in range(iters):
        t0 = _time.perf_counter()
        last = None
        for _ in range(inner):
            last = sharded(*dev_args)
        for o in last:
            o.block_until_ready()
        times.append((_time.perf_counter() - t0) / inner)
    return times

